# revision 66
# baseline (speedup 1.0000x reference)
"""CrossBatchAttention Trainium2 kernel — 8-core tensor-parallel SPMD.

v6: fp8 DoubleRow everywhere; collective-lean schedule.

  - KVQ pass computes K, V and the gate-W1 X-part (g1X, full 4096
    contraction for this core's 128 gate-hidden rows) for quarters in
    order 1,2,3,0, then Q for quarter 0 from the still-resident X tile.
    Remaining Q projections run inside the block loop (one m-tile per
    block), stretching block-phase PE time well past the collective
    core's serial time so AllGathers never become the pacer.
  - One OT AllGather per 512-query chunk (all 4 local heads, 256KB in
    -> 2MB out) in natural head order: no Wo permutation; out_proj and
    the fused gate matmul read the gathered buffer directly.
  - Gate hidden = gelu(gWf^T @ otg + g1X + b1) where gWf = Wo @ gW1c is
    fused on the host; no ReduceScatter anywhere.  Only a small
    per-chunk AllGather of the activated 128-row shard remains.
  - Chunk 3 splits its sharing pipeline: pair A (heads 0,1) gathers
    wide after block 13 and is consumed under the wire time of the two
    half-width (256-col) pair-B AllGathers, so the tail chain after the
    last attention block is short.

Quantization: X fp8, W* fp8 x64 (/64 on PSUM exit), qt/kt bf16,
P = exp(s/sqrt(d) - 5.0) fp8, ones = 1/8 so rec = 8/den, otc = O*8
fp8, cacc fp8, g1X fp8 true-scale, sigmoid(logits/64).  Host: concat
8 [512,2048] bf16 shards, transpose, add X -> f32.
"""

import numpy as np
import ml_dtypes

import concourse.bass as bass
import concourse.bass_isa as bass_isa
import concourse.mybir as mybir
import concourse.tile as tile
from concourse import bacc
from concourse import bass_utils

BF16 = mybir.dt.bfloat16
F32 = mybir.dt.float32
F8 = mybir.dt.float8e4
DR = mybir.MatmulPerfMode.DoubleRow
W_SCALE = 64.0           # all fp8 weights scaled by this on host
O_SCALE = 8.0            # otc = O * 8 (via ones=1/8 in denominator)
EBIAS = -5.0             # exp(s*SCALE + EBIAS): keeps P in fp8 range

B = 2048
HID = 4096
NH = 32
HD = 128
GH = 1024
NC_ = 8
HPC = NH // NC_          # heads per core = 4
HS = HID // NC_          # hid shard = 512
GS = GH // NC_           # gate-hidden shard = 128
SCALE = 1.0 / float(np.sqrt(HD))

KT_TILES = HID // 128    # 32 k-tiles over the 4096 contraction
KP = KT_TILES // 2       # 16 DoubleRow k-steps
JT = B // 128            # 16 j-tiles over keys
JP = JT // 2             # 8 DoubleRow j-steps
IC = B // 512            # 4 i-chunks of 512 over batch

GELU_FUNC = mybir.ActivationFunctionType.Gelu


def _build_program(allones: bool):
    nc = bacc.Bacc(
        "TRN2",
        target_bir_lowering=False,
        debug=False,
        enable_asserts=False,
        num_devices=NC_,
    )

    # ---- I/O declarations (per-core shapes) ----
    xt_d = nc.dram_tensor("xt", [128, IC, KT_TILES, 512], F8, kind="ExternalInput").ap()
    wq_d = nc.dram_tensor("wq", [128, KT_TILES, HS], F8, kind="ExternalInput").ap()
    wk_d = nc.dram_tensor("wk", [128, KT_TILES, HS], F8, kind="ExternalInput").ap()
    wv_d = nc.dram_tensor("wv", [128, KT_TILES, HS], F8, kind="ExternalInput").ap()
    wo_d = nc.dram_tensor("wo", [128, KT_TILES, HS], F8, kind="ExternalInput").ap()
    gw1x_d = nc.dram_tensor("gw1x", [128, KT_TILES, GS], F8, kind="ExternalInput").ap()
    gwf_d = nc.dram_tensor("gwf", [128, KT_TILES, GS], F8, kind="ExternalInput").ap()
    gw2_d = nc.dram_tensor("gw2", [128, NC_, HS], F8, kind="ExternalInput").ap()
    gb1_d = nc.dram_tensor("gb1", [GS, 1], F32, kind="ExternalInput").ap()
    gb2_d = nc.dram_tensor("gb2", [128, 4], F32, kind="ExternalInput").ap()
    mask01_d = nc.dram_tensor("mask01", [128, JT], BF16, kind="ExternalInput").ap()
    diagm_d = nc.dram_tensor("diagm", [128, 128], F8, kind="ExternalInput").ap()
    out_d = nc.dram_tensor("out", [HS, B], BF16, kind="ExternalOutput").ap()

    groups = [list(range(NC_))]

    with tile.TileContext(nc) as tc:
        with (
            tc.tile_pool(name="persist", bufs=1) as persist,
            tc.tile_pool(name="psum", bufs=1, space="PSUM") as psum,
            tc.tile_pool(name="dram", bufs=1, space="DRAM") as dram,
        ):
            # ---------- persistent SBUF ----------
            kt_sb = persist.tile([128, HPC, B], BF16)     # [d, head, j] 2MB
            v_sb = persist.tile([128, JT, HS], F8)        # [j_in, j_tile, hd] 1MB
            mask01_sb = persist.tile([128, JT], BF16)
            diagm_sb = persist.tile([128, 128], F8)
            ones_sb = persist.tile([128, 2, 128], F8)
            ebias_sb = persist.tile([128, 1], F32)
            gb1_sb = persist.tile([GS, 1], F32)
            gb2_sb = persist.tile([128, 4], F32)
            wq_sb = persist.tile([128, KT_TILES, HS], F8)     # 2MB
            wo_sb = persist.tile([128, KT_TILES, HS], F8)     # 2MB
            gw1x_sb = persist.tile([128, KT_TILES, GS], F8)
            gwf_sb = persist.tile([128, KT_TILES, GS], F8)
            gw2_sb = persist.tile([128, NC_, HS], F8)
            cacc = persist.tile([128, 4, B], F8)
            g1x_sb = persist.tile([128, B], F8)           # gW1x^T X, gh shard

            nc.vector.memset(ones_sb, 1.0 / O_SCALE)
            nc.vector.memset(ebias_sb, EBIAS)

            # ---------- DRAM bounce buffers for collectives ----------
            # chunks 0..2: one AG per chunk, natural head order
            # AG inputs are written j-major ([j, head, i]) so the gathered
            # output clusters into per-partition-contiguous 2KB chunks and
            # the otg reload's descriptor generation stays cheap.
            ag_in_c, ag_out_c = [], []
            for q in range(IC - 1):
                ag_in_c.append(dram.tile([128, HPC, 512], F8,
                                         name=f"ag_in{q}"))
                ag_out_c.append(dram.tile([NC_ * 512, 512], F8,
                                          addr_space="Shared",
                                          name=f"ag_out{q}"))
            # chunk 3: pair A (heads 0,1) wide; pair B (heads 2,3) in halves
            ag_a3_in = dram.tile([128, 2, 512], F8, name="ag_a3_in")
            ag_a3_out = dram.tile([NC_ * 256, 512], F8, addr_space="Shared",
                                  name="ag_a3_out")
            ag_b3_in = dram.tile([128, 2, 512], F8, name="ag_b3_in")
            ag_b3_out = dram.tile([NC_ * 256, 512], F8, addr_space="Shared",
                                  name="ag_b3_out")
            ag2_in_c, ag2_out_c = [], []
            for icc in range(IC):
                ag2_in_c.append(dram.tile([GS, 512], F8, name=f"ag2_in{icc}"))
                ag2_out_c.append(dram.tile([GH, 512], F8, addr_space="Shared",
                                           name=f"ag2_out{icc}"))

            # one tiny warmup to kick off ncfw init (~100us) early; per-shape
            # warmups measurably do nothing for later collectives' speed
            warm_i = dram.tile([1, 512], F8, name="warm_i")
            warm_o = dram.tile([NC_, 512], F8, addr_space="Shared",
                               name="warm_o")
            nc.gpsimd.collective_compute(
                "AllGather", mybir.AluOpType.bypass, replica_groups=groups,
                ins=[warm_i.opt()], outs=[warm_o.opt()],
            )

            with tc.tile_pool(name="main", bufs=1) as mp:
                def load_xt(q):
                    xt_q = mp.tile([128, KT_TILES, 512], F8, tag="xtb",
                                   bufs=2, name="xt_q")
                    nc.sync.dma_start(out=xt_q, in_=xt_d[:, q])
                    return xt_q

                def proj_dr(wsb, msl, xt_q, ps):
                    for k in range(KP):
                        nc.tensor.matmul(
                            ps,
                            lhsT=wsb[:, 2 * k:2 * k + 2, msl],
                            rhs=xt_q[:, 2 * k:2 * k + 2, :],
                            start=(k == 0),
                            stop=(k == KP - 1),
                            perf_mode=DR,
                        )

                # ======== K/V/g1X pass (quarters 1,2,3,0) + Q(0) ========
                with tc.tile_pool(name="pkv", bufs=1) as pkv:
                    wk_sb = pkv.tile([128, KT_TILES, HS], F8, tag="wk", bufs=1)
                    wv_sb = pkv.tile([128, KT_TILES, HS], F8, tag="wv", bufs=1)

                    # first quarter (q=1) X and Wk in 4-k-tile chunks so the
                    # k-outer loop starts as soon as the first 256KB lands
                    xt_first = mp.tile([128, KT_TILES, 512], F8, tag="xtb",
                                       bufs=2, name="xt_q")
                    NCH = 8
                    for ch in range(NCH):
                        ksl = slice(ch * 4, (ch + 1) * 4)
                        nc.sync.dma_start(out=xt_first[:, ksl, :],
                                          in_=xt_d[:, 1, ksl, :])
                        nc.sync.dma_start(out=wk_sb[:, ksl, :],
                                          in_=wk_d[:, ksl, :])
                    xt_next = load_xt(2)

                    # weights on the scalar DMA queue
                    nc.scalar.dma_start(out=wv_sb, in_=wv_d)
                    nc.scalar.dma_start(out=wq_sb, in_=wq_d)
                    nc.scalar.dma_start(out=gw1x_sb, in_=gw1x_d)
                    nc.scalar.dma_start(out=wo_sb, in_=wo_d)
                    nc.scalar.dma_start(out=gwf_sb, in_=gwf_d)
                    nc.scalar.dma_start(out=gw2_sb, in_=gw2_d)
                    if not allones:
                        nc.scalar.dma_start(out=mask01_sb, in_=mask01_d)
                    nc.scalar.dma_start(out=diagm_sb, in_=diagm_d)
                    nc.scalar.dma_start(out=gb1_sb, in_=gb1_d)
                    nc.scalar.dma_start(out=gb2_sb, in_=gb2_d)

                    def v_pass(q, xt_q):
                        for it in range(4):
                            ps = psum.tile([128, 512], F32, tag="mm", bufs=2,
                                           name="ps_v")
                            for k in range(KP):
                                nc.tensor.matmul(
                                    ps,
                                    lhsT=xt_q[:, 2 * k:2 * k + 2,
                                              it * 128:(it + 1) * 128],
                                    rhs=wv_sb[:, 2 * k:2 * k + 2, :],
                                    start=(k == 0),
                                    stop=(k == KP - 1),
                                    perf_mode=DR,
                                )
                            nc.vector.tensor_scalar_mul(
                                v_sb[:, q * 4 + it, :], ps, 1.0 / W_SCALE
                            )

                    def g1x_pass(q, xt_q):
                        isl = slice(q * 512, (q + 1) * 512)
                        ps = psum.tile([128, 512], F32, tag="mm", bufs=2,
                                       name="ps_g1x")
                        proj_dr(gw1x_sb, slice(0, GS), xt_q, ps)
                        nc.vector.tensor_scalar_mul(
                            g1x_sb[:, isl], ps, 1.0 / W_SCALE
                        )

                    # quarter 1: k-outer K pass over 4 live PSUM banks
                    # (borrowing the scores' "st" tag banks)
                    kpsA = psum.tile([128, 2, 512], F32, tag="st", bufs=2,
                                     name="kpsA")
                    kpsB = psum.tile([128, 2, 512], F32, tag="st", bufs=2,
                                     name="kpsB")
                    kps = [kpsA[:, 0, :], kpsA[:, 1, :],
                           kpsB[:, 0, :], kpsB[:, 1, :]]
                    for ch in range(NCH):
                        for m in range(4):
                            for u in range(2):
                                nc.tensor.matmul(
                                    kps[m],
                                    lhsT=wk_sb[:, 4 * ch + 2 * u:
                                               4 * ch + 2 * u + 2,
                                               m * 128:(m + 1) * 128],
                                    rhs=xt_first[:, 4 * ch + 2 * u:
                                                 4 * ch + 2 * u + 2, :],
                                    start=(ch == 0 and u == 0),
                                    stop=(ch == NCH - 1 and u == 1),
                                    perf_mode=DR,
                                )
                    for m in range(4):
                        nc.vector.tensor_scalar_mul(
                            kt_sb[:, m, 512:1024], kps[m], 1.0 / W_SCALE
                        )
                    v_pass(1, xt_first)
                    g1x_pass(1, xt_first)

                    xt_last = None
                    for q in (2, 3, 0):
                        xt_q = xt_next
                        if q == 2:
                            xt_next = load_xt(3)
                        elif q == 3:
                            xt_next = load_xt(0)
                        isl = slice(q * 512, (q + 1) * 512)
                        for m in range(4):
                            ps = psum.tile([128, 512], F32, tag="mm", bufs=2,
                                           name="ps_k")
                            proj_dr(wk_sb, slice(m * 128, (m + 1) * 128),
                                    xt_q, ps)
                            nc.vector.tensor_scalar_mul(
                                kt_sb[:, m, isl], ps, 1.0 / W_SCALE
                            )
                        v_pass(q, xt_q)
                        g1x_pass(q, xt_q)
                        xt_last = xt_q

                    # stream next-quarter X for the in-loop Q projections
                    xt_blk = {1: load_xt(1)}
                    # Q(0) from the still-resident quarter-0 tile
                    qt_cur = mp.tile([128, HPC, 512], BF16, tag="qt", bufs=2,
                                     name="qt")
                    for m in range(4):
                        ps = psum.tile([128, 512], F32, tag="mm", bufs=2,
                                       name="ps_q")
                        proj_dr(wq_sb, slice(m * 128, (m + 1) * 128),
                                xt_last, ps)
                        nc.vector.tensor_scalar_mul(
                            qt_cur[:, m, :], ps, 1.0 / W_SCALE
                        )

                # ======== interleaved block phase ========

                def attention_block(h, q, qt):
                    pt = mp.tile([128, JT, 512], F8, tag="pt", bufs=2,
                                 name="pt")
                    for jp in range(JP):
                        st = psum.tile([128, 2, 512], F32, tag="st",
                                       bufs=2, name="st")
                        for uu in range(2):
                            j = 2 * jp + uu
                            nc.tensor.matmul(
                                st[:, uu, :],
                                lhsT=kt_sb[:, h, j * 128:(j + 1) * 128],
                                rhs=qt[:, h, :],
                                start=True,
                                stop=True,
                            )
                        nc.scalar.activation(
                            pt[:, 2 * jp:2 * jp + 2, :],
                            st,
                            mybir.ActivationFunctionType.Exp,
                            bias=ebias_sb,
                            scale=SCALE,
                        )
                        for uu in range(2):
                            j = 2 * jp + uu
                            if not allones:
                                nc.vector.tensor_scalar_mul(
                                    pt[:, j, :], pt[:, j, :],
                                    mask01_sb[:, j:j + 1],
                                )
                            if j // 4 == q:
                                c0 = (j % 4) * 128
                                nc.vector.tensor_mul(
                                    pt[:, j, c0:c0 + 128],
                                    pt[:, j, c0:c0 + 128],
                                    diagm_sb,
                                )
                    den_ps = psum.tile([128, 512], F32, tag="acc", bufs=2,
                                       name="den_ps")
                    for jp in range(JP):
                        nc.tensor.matmul(
                            den_ps,
                            lhsT=ones_sb,
                            rhs=pt[:, 2 * jp:2 * jp + 2, :],
                            start=(jp == 0),
                            stop=(jp == JP - 1),
                            perf_mode=DR,
                        )
                    ot_ps = psum.tile([128, 512], F32, tag="acc", bufs=2,
                                      name="ot_ps")
                    for jp in range(JP):
                        nc.tensor.matmul(
                            ot_ps,
                            lhsT=v_sb[:, 2 * jp:2 * jp + 2,
                                      h * 128:(h + 1) * 128],
                            rhs=pt[:, 2 * jp:2 * jp + 2, :],
                            start=(jp == 0),
                            stop=(jp == JP - 1),
                            perf_mode=DR,
                        )
                    rec = mp.tile([128, 512], F32, tag="rec", bufs=1)
                    nc.vector.reciprocal_approx_fast(out=rec, in_=den_ps)
                    otc = mp.tile([128, 512], F8, tag="otc", bufs=1)
                    nc.vector.tensor_mul(otc, ot_ps, rec)
                    if q < IC - 1:
                        nc.sync.dma_start(out=ag_in_c[q][:, h, :], in_=otc)
                        if h == 3:
                            nc.gpsimd.collective_compute(
                                "AllGather", mybir.AluOpType.bypass,
                                replica_groups=groups,
                                ins=[ag_in_c[q].opt()],
                                outs=[ag_out_c[q].opt()],
                            )
                    elif h < 2:
                        nc.sync.dma_start(out=ag_a3_in[:, h, :], in_=otc)
                        if h == 1:
                            nc.gpsimd.collective_compute(
                                "AllGather", mybir.AluOpType.bypass,
                                replica_groups=groups,
                                ins=[ag_a3_in.opt()],
                                outs=[ag_a3_out.opt()],
                            )
                    else:
                        nc.sync.dma_start(out=ag_b3_in[:, h - 2, :], in_=otc)
                        if h == 3:
                            nc.gpsimd.collective_compute(
                                "AllGather", mybir.AluOpType.bypass,
                                replica_groups=groups,
                                ins=[ag_b3_in.opt()],
                                outs=[ag_b3_out.opt()],
                            )

                def otg_load(ic):
                    otg = mp.tile([128, NC_, HPC, 512], F8, tag="otg", bufs=1,
                                  name="otg")
                    nc.sync.dma_start(
                        out=otg,
                        in_=ag_out_c[ic].rearrange("(r j h) i -> j r h i",
                                                   r=NC_, j=128, h=HPC),
                    )
                    return otg

                def otg_pair(otg, t):
                    # head-tile pair (2t, 2t+1) of a 4-D gathered buffer
                    return otg[:, t // 2, 2 * (t % 2):2 * (t % 2) + 2, :]

                def outproj_chunk(ic, otg):
                    csl = slice(ic * 512, (ic + 1) * 512)
                    for m in range(4):
                        ps = psum.tile([128, 512], F32, tag="mm", bufs=2,
                                       name="ps_wo")
                        for r in range(KP):
                            nc.tensor.matmul(
                                ps,
                                lhsT=wo_sb[:, 2 * r:2 * r + 2,
                                           m * 128:(m + 1) * 128],
                                rhs=otg_pair(otg, r),
                                start=(r == 0),
                                stop=(r == KP - 1),
                                perf_mode=DR,
                            )
                        nc.vector.tensor_scalar_mul(
                            cacc[:, m, csl], ps, 1.0 / (W_SCALE * O_SCALE)
                        )

                def g1_finish(pre, w, ag2i, ag2o):
                    gt_ch = mp.tile([128, w], F8, tag="gt", bufs=1)
                    nc.scalar.activation(gt_ch, pre, GELU_FUNC,
                                         bias=gb1_sb, scale=1.0)
                    nc.sync.dma_start(out=ag2i, in_=gt_ch)
                    nc.gpsimd.collective_compute(
                        "AllGather", mybir.AluOpType.bypass,
                        replica_groups=groups,
                        ins=[ag2i.opt()], outs=[ag2o.opt()],
                    )

                def g1_chunk(ic, otg):
                    csl = slice(ic * 512, (ic + 1) * 512)
                    ps = psum.tile([128, 512], F32, tag="mm", bufs=2,
                                   name="ps_g1")
                    for r in range(KP):
                        nc.tensor.matmul(
                            ps,
                            lhsT=gwf_sb[:, 2 * r:2 * r + 2, :],
                            rhs=otg_pair(otg, r),
                            start=(r == 0),
                            stop=(r == KP - 1),
                            perf_mode=DR,
                        )
                    g1pre = mp.tile([128, 512], BF16, tag="g1pre", bufs=1)
                    nc.vector.scalar_tensor_tensor(
                        g1pre, ps, 1.0 / (W_SCALE * O_SCALE),
                        g1x_sb[:, csl],
                        op0=mybir.AluOpType.mult,
                        op1=mybir.AluOpType.add,
                    )
                    g1_finish(g1pre, 512, ag2_in_c[ic], ag2_out_c[ic])

                def gtf_load(ic):
                    gtf = mp.tile([128, NC_, 512], F8, tag="gtf",
                                  bufs=1, name="gtf")
                    nc.sync.dma_start(
                        out=gtf,
                        in_=ag2_out_c[ic].rearrange("(r p) i -> p r i", p=128),
                    )
                    return gtf

                def gate_chain(c0, w, gtf):
                    csl = slice(c0, c0 + w)
                    for m in range(4):
                        ps = psum.tile([128, w], F32, tag="mm", bufs=2,
                                       name="ps_gw2")
                        for r in range(NC_ // 2):
                            nc.tensor.matmul(
                                ps,
                                lhsT=gw2_sb[:, 2 * r:2 * r + 2,
                                            m * 128:(m + 1) * 128],
                                rhs=gtf[:, 2 * r:2 * r + 2, :],
                                start=(r == 0),
                                stop=(r == NC_ // 2 - 1),
                                perf_mode=DR,
                            )
                        gate_ch = mp.tile([128, w], BF16, tag="gate",
                                          bufs=1)
                        nc.scalar.activation(
                            gate_ch, ps,
                            mybir.ActivationFunctionType.Sigmoid,
                            bias=gb2_sb[:, m:m + 1], scale=1.0 / W_SCALE,
                        )
                        outt = mp.tile([128, w], BF16, tag="outt", bufs=1)
                        nc.vector.tensor_mul(outt, gate_ch, cacc[:, m, csl])
                        nc.sync.dma_start(
                            out=out_d[m * 128:(m + 1) * 128, csl], in_=outt
                        )

                def qproj_blk(m, xtb, qt_next):
                    ps = psum.tile([128, 512], F32, tag="mm", bufs=2,
                                   name="ps_q")
                    proj_dr(wq_sb, slice(m * 128, (m + 1) * 128), xtb, ps)
                    nc.vector.tensor_scalar_mul(
                        qt_next[:, m, :], ps, 1.0 / W_SCALE
                    )

                # schedule (chunk-AG ~29us ~ 1.6 blocks):
                #   AG(ic) trigger @ 4ic+3 (in attention)
                #   otg(ic) @ 4ic+5   g1(ic) @ 4ic+6-pre  outproj(ic) @ 4ic+6
                #   gtf(ic) @ 4ic+8   gate(ic) @ 4ic+9
                otg_pend = {}
                gtf_pend = {}
                qt_next = None
                otg_a3 = None
                for s in range(16):
                    h, q = s % 4, s // 4
                    r4 = s % 4
                    attention_block(h, q, qt_cur)
                    if r4 == 0 and q + 1 < IC:
                        qt_next = mp.tile([128, HPC, 512], BF16, tag="qt",
                                          bufs=2, name="qt")
                        if q + 2 < IC:
                            xt_blk[q + 2] = load_xt(q + 2)
                    # chunk 0's consumers lag 2 extra blocks: the first AG
                    # absorbs the cross-core skew accumulated over the
                    # projection pass (~25us)
                    if s in (8, 9, 13):
                        ic = 0 if s == 8 else (s - 5) // 4
                        otg_pend[ic] = otg_load(ic)
                    if s in (9, 10, 14):
                        ic = 0 if s == 9 else (s - 6) // 4
                        g1_chunk(ic, otg_pend[ic])
                        outproj_chunk(ic, otg_pend.pop(ic))
                    if s in (11, 12):
                        ic = 0 if s == 11 else (s - 8) // 4
                        gtf_pend[ic] = gtf_load(ic)
                    if s in (12, 13):
                        ic = 0 if s == 12 else (s - 9) // 4
                        gate_chain(ic * 512, 512, gtf_pend.pop(ic))
                    if s == 14:
                        # pair-A gather of chunk 3 (triggered end of s13)
                        otg_a3 = mp.tile([128, NC_, 2, 512], F8, tag="otg",
                                         bufs=1, name="otga3")
                        nc.sync.dma_start(
                            out=otg_a3,
                            in_=ag_a3_out.rearrange("(r j u) i -> j r u i",
                                                    r=NC_, j=128, u=2),
                        )
                    if q + 1 < IC:
                        qproj_blk(h, xt_blk[q + 1], qt_next)
                    if r4 == 3 and q + 1 < IC:
                        qt_cur = qt_next

                # ---- tail: chunk 3 (pair A wide, pair B split in halves) ----
                # pair-A out_proj and g1 A-parts run under the pair-B AG wire
                for m in range(4):
                    ps = psum.tile([128, 512], F32, tag="mm", bufs=2,
                                   name="ps_wo")
                    for r in range(NC_):
                        nc.tensor.matmul(
                            ps,
                            lhsT=wo_sb[:, 4 * r:4 * r + 2,
                                       m * 128:(m + 1) * 128],
                            rhs=otg_a3[:, r, :, :],
                            start=(r == 0),
                            stop=(r == NC_ - 1),
                            perf_mode=DR,
                        )
                    nc.vector.tensor_scalar_mul(
                        cacc[:, m, 1536:2048], ps, 1.0 / (W_SCALE * O_SCALE)
                    )
                psA = psum.tile([128, 512], F32, tag="mm", bufs=2,
                                name="ps_g1a")
                for r in range(NC_):
                    nc.tensor.matmul(
                        psA,
                        lhsT=gwf_sb[:, 4 * r:4 * r + 2, :],
                        rhs=otg_a3[:, r, :, :],
                        start=(r == 0),
                        stop=(r == NC_ - 1),
                        perf_mode=DR,
                    )
                tmpA = mp.tile([128, 512], BF16, tag="g1tmp", bufs=1)
                nc.vector.scalar_tensor_tensor(
                    tmpA, psA, 1.0 / (W_SCALE * O_SCALE),
                    g1x_sb[:, 1536:2048],
                    op0=mybir.AluOpType.mult,
                    op1=mybir.AluOpType.add,
                )
                gtf_pend[2] = gtf_load(2)
                gate_chain(1024, 512, gtf_pend.pop(2))
                otg_b = mp.tile([128, NC_, 2, 512], F8, tag="otg",
                                bufs=1, name="otgb3")
                nc.sync.dma_start(
                    out=otg_b,
                    in_=ag_b3_out.rearrange("(r j u) i -> j r u i",
                                            r=NC_, j=128, u=2),
                )
                # g1 B-part first: it gates the AG2(3) trigger
                psb = psum.tile([128, 512], F32, tag="mm", bufs=2,
                                name="ps_g1b")
                for r in range(NC_):
                    nc.tensor.matmul(
                        psb,
                        lhsT=gwf_sb[:, 4 * r + 2:4 * r + 4, :],
                        rhs=otg_b[:, r, :, :],
                        start=(r == 0),
                        stop=(r == NC_ - 1),
                        perf_mode=DR,
                    )
                g1pre = mp.tile([128, 512], BF16, tag="g1pre", bufs=1)
                nc.vector.scalar_tensor_tensor(
                    g1pre, psb, 1.0 / (W_SCALE * O_SCALE), tmpA,
                    op0=mybir.AluOpType.mult,
                    op1=mybir.AluOpType.add,
                )
                g1_finish(g1pre, 512, ag2_in_c[3], ag2_out_c[3])
                for m in range(4):
                    ps = psum.tile([128, 512], F32, tag="mm", bufs=2,
                                   name="ps_wob")
                    for r in range(NC_):
                        nc.tensor.matmul(
                            ps,
                            lhsT=wo_sb[:, 4 * r + 2:4 * r + 4,
                                       m * 128:(m + 1) * 128],
                            rhs=otg_b[:, r, :, :],
                            start=(r == 0),
                            stop=(r == NC_ - 1),
                            perf_mode=DR,
                        )
                    nc.vector.scalar_tensor_tensor(
                        cacc[:, m, 1536:2048], ps,
                        1.0 / (W_SCALE * O_SCALE),
                        cacc[:, m, 1536:2048],
                        op0=mybir.AluOpType.mult,
                        op1=mybir.AluOpType.add,
                    )
                gtf3 = gtf_load(3)
                gate_chain(1536, 512, gtf3)

    nc.compile()
    return nc


def _q8(x, scale=1.0):
    f8 = ml_dtypes.float8_e4m3
    return np.ascontiguousarray(
        np.clip(np.asarray(x, dtype=np.float32) * scale, -240.0, 240.0)
    ).astype(f8)


def _make_in_maps(inputs):
    f32 = np.float32
    X = np.asarray(inputs["hidden_states"], dtype=f32)
    mask = np.asarray(inputs["attention_mask"])
    Wq = np.asarray(inputs["Wq"], dtype=f32)
    Wk = np.asarray(inputs["Wk"], dtype=f32)
    Wv = np.asarray(inputs["Wv"], dtype=f32)
    Wo = np.asarray(inputs["Wo"], dtype=f32)
    gW1 = np.asarray(inputs["gW1"], dtype=f32)
    gb1 = np.asarray(inputs["gb1"], dtype=f32)
    gW2 = np.asarray(inputs["gW2"], dtype=f32)
    gb2 = np.asarray(inputs["gb2"], dtype=f32)

    XT8 = _q8(X.T)                                       # [4096, 2048]
    XTT = np.ascontiguousarray(
        XT8.reshape(KT_TILES, 128, IC, 512).transpose(1, 2, 0, 3))

    def _tile_w(w8):  # [K, M] -> [128, K/128, M]
        kt = w8.shape[0] // 128
        return np.ascontiguousarray(
            w8.reshape(kt, 128, w8.shape[1]).transpose(1, 0, 2))

    mask01_t = np.ascontiguousarray(
        mask.astype(f32).reshape(JT, 128).T).astype(ml_dtypes.bfloat16)
    diagm = _q8(1.0 - np.eye(128, dtype=f32))

    # fused Wo @ gW1c: attention-output features (natural head order) -> gh
    Wf = Wo @ gW1[HID:]                                  # [4096, 1024]
    gW1x = gW1[:HID]                                     # [4096, 1024]

    in_maps = []
    for c in range(NC_):
        hsl = slice(c * HS, (c + 1) * HS)
        gsl = slice(c * GS, (c + 1) * GS)
        in_maps.append({
            "xt": XTT,
            "wq": _tile_w(_q8(Wq[:, hsl], W_SCALE)),
            "wk": _tile_w(_q8(Wk[:, hsl], W_SCALE)),
            "wv": _tile_w(_q8(Wv[:, hsl], W_SCALE)),
            "wo": _tile_w(_q8(Wo[:, hsl], W_SCALE)),
            "gw1x": _tile_w(_q8(gW1x[:, gsl], W_SCALE)),
            "gwf": _tile_w(_q8(Wf[:, gsl], W_SCALE)),
            "gw2": _tile_w(_q8(gW2[:, hsl], W_SCALE)),
            "gb1": np.ascontiguousarray(gb1[gsl].reshape(GS, 1)),
            "gb2": np.ascontiguousarray(gb2[hsl].reshape(4, 128).T),
            "mask01": mask01_t,
            "diagm": diagm,
        })
    return in_maps


_NC_CACHE = {}


def _run(inputs, trace=False):
    allones = bool(np.asarray(inputs["attention_mask"]).all())
    nc = _NC_CACHE.get(allones)
    if nc is None:
        nc = _build_program(allones)
        _NC_CACHE[allones] = nc
    in_maps = _make_in_maps(inputs)
    res = bass_utils.run_bass_kernel_spmd(
        nc, in_maps, core_ids=list(range(NC_)), trace=trace
    )
    shards = [np.asarray(res.results[c]["out"], dtype=np.float32)
              for c in range(NC_)]
    gated = np.concatenate(shards, axis=0).T  # gate * cross, [2048, 4096]
    out = np.asarray(inputs["hidden_states"], dtype=np.float32) + gated
    return np.ascontiguousarray(out), res


def kernel(**inputs) -> np.ndarray:
    out, _ = _run(inputs, trace=False)
    return out


# revision 67
# speedup vs baseline: 1.0599x; 1.0599x over previous
"""CrossBatchAttention Trainium2 kernel — 8-core tensor-parallel SPMD.

v6: fp8 DoubleRow everywhere; collective-lean schedule.

  - KVQ pass computes K, V and the gate-W1 X-part (g1X, full 4096
    contraction for this core's 128 gate-hidden rows) for quarters in
    order 1,2,3,0, then Q for quarter 0 from the still-resident X tile.
    Remaining Q projections run inside the block loop (one m-tile per
    block), stretching block-phase PE time well past the collective
    core's serial time so AllGathers never become the pacer.
  - One OT AllGather per 512-query chunk (all 4 local heads, 256KB in
    -> 2MB out) in natural head order: no Wo permutation; out_proj and
    the fused gate matmul read the gathered buffer directly.
  - Gate hidden = gelu(gWf^T @ otg + g1X + b1) where gWf = Wo @ gW1c is
    fused on the host; no ReduceScatter anywhere.  Only a small
    per-chunk AllGather of the activated 128-row shard remains.
  - Chunk 3 splits its sharing pipeline: pair A (heads 0,1) gathers
    wide after block 13 and is consumed under the wire time of the two
    half-width (256-col) pair-B AllGathers, so the tail chain after the
    last attention block is short.

Quantization: X fp8, W* fp8 x64 (/64 on PSUM exit), qt/kt bf16,
P = exp(s/sqrt(d) - 5.0) fp8, ones = 1/8 so rec = 8/den, otc = O*8
fp8, cacc fp8, g1X fp8 true-scale, sigmoid(logits/64).  Host: concat
8 [512,2048] bf16 shards, transpose, add X -> f32.
"""

import numpy as np
import ml_dtypes

import concourse.bass as bass
import concourse.mybir as mybir
import concourse.tile as tile
from concourse import bacc
from concourse import bass_utils

BF16 = mybir.dt.bfloat16
F32 = mybir.dt.float32
F8 = mybir.dt.float8e4
DR = mybir.MatmulPerfMode.DoubleRow
W_SCALE = 64.0           # all fp8 weights scaled by this on host
O_SCALE = 8.0            # otc = O * 8 (via ones=1/8 in denominator)
EBIAS = -5.0             # exp(s*SCALE + EBIAS): keeps P in fp8 range

B = 2048
HID = 4096
NH = 32
HD = 128
GH = 1024
NC_ = 8
HPC = NH // NC_          # heads per core = 4
HS = HID // NC_          # hid shard = 512
GS = GH // NC_           # gate-hidden shard = 128
SCALE = 1.0 / float(np.sqrt(HD))

KT_TILES = HID // 128    # 32 k-tiles over the 4096 contraction
KP = KT_TILES // 2       # 16 DoubleRow k-steps
JT = B // 128            # 16 j-tiles over keys
JP = JT // 2             # 8 DoubleRow j-steps
IC = B // 512            # 4 i-chunks of 512 over batch

GELU_FUNC = mybir.ActivationFunctionType.Gelu


def _build_program(allones: bool):
    nc = bacc.Bacc(
        "TRN2",
        target_bir_lowering=False,
        debug=False,
        enable_asserts=False,
        num_devices=NC_,
    )

    # ---- I/O declarations (per-core shapes) ----
    xt_d = nc.dram_tensor("xt", [128, IC, KT_TILES, 512], F8, kind="ExternalInput").ap()
    wq_d = nc.dram_tensor("wq", [128, KT_TILES, HS], F8, kind="ExternalInput").ap()
    wk_d = nc.dram_tensor("wk", [128, KT_TILES, HS], F8, kind="ExternalInput").ap()
    wv_d = nc.dram_tensor("wv", [128, KT_TILES, HS], F8, kind="ExternalInput").ap()
    wo_d = nc.dram_tensor("wo", [128, KT_TILES, HS], F8, kind="ExternalInput").ap()
    gw1x_d = nc.dram_tensor("gw1x", [128, KT_TILES, GS], F8, kind="ExternalInput").ap()
    gwf_d = nc.dram_tensor("gwf", [128, KT_TILES, GS], F8, kind="ExternalInput").ap()
    gw2_d = nc.dram_tensor("gw2", [128, NC_, HS], F8, kind="ExternalInput").ap()
    gb1_d = nc.dram_tensor("gb1", [GS, 1], F32, kind="ExternalInput").ap()
    gb2_d = nc.dram_tensor("gb2", [128, 4], F32, kind="ExternalInput").ap()
    mask01_d = nc.dram_tensor("mask01", [128, JT], BF16, kind="ExternalInput").ap()
    diagm_d = nc.dram_tensor("diagm", [128, 128], F8, kind="ExternalInput").ap()
    out_d = nc.dram_tensor("out", [HS, B], BF16, kind="ExternalOutput").ap()

    groups = [list(range(NC_))]

    with tile.TileContext(nc) as tc:
        with (
            tc.tile_pool(name="persist", bufs=1) as persist,
            tc.tile_pool(name="psum", bufs=1, space="PSUM") as psum,
            tc.tile_pool(name="dram", bufs=1, space="DRAM") as dram,
        ):
            # ---------- persistent SBUF ----------
            kt_sb = persist.tile([128, HPC, B], BF16)     # [d, head, j] 2MB
            v_sb = persist.tile([128, JT, HS], F8)        # [j_in, j_tile, hd] 1MB
            mask01_sb = persist.tile([128, JT], BF16)
            diagm_sb = persist.tile([128, 128], F8)
            ones_sb = persist.tile([128, 2, 128], F8)
            ebias_sb = persist.tile([128, 1], F32)
            gb1_sb = persist.tile([GS, 1], F32)
            gb2_sb = persist.tile([128, 4], F32)
            wq_sb = persist.tile([128, KT_TILES, HS], F8)     # 2MB
            wo_sb = persist.tile([128, KT_TILES, HS], F8)     # 2MB
            gw1x_sb = persist.tile([128, KT_TILES, GS], F8)
            gwf_sb = persist.tile([128, KT_TILES, GS], F8)
            gw2_sb = persist.tile([128, NC_, HS], F8)
            cacc = persist.tile([128, 4, B], F8)
            g1x_sb = persist.tile([128, B], F8)           # gW1x^T X, gh shard

            nc.vector.memset(ones_sb, 1.0 / O_SCALE)
            nc.vector.memset(ebias_sb, EBIAS)

            # ---------- DRAM bounce buffers for collectives ----------
            # chunks 0..2: one AG per chunk, natural head order
            # AG inputs are written j-major ([j, head, i]) so the gathered
            # output clusters into per-partition-contiguous 2KB chunks and
            # the otg reload's descriptor generation stays cheap.
            ag_in_c, ag_out_c = [], []
            for q in range(IC - 1):
                ag_in_c.append(dram.tile([128, HPC, 512], F8,
                                         name=f"ag_in{q}"))
                ag_out_c.append(dram.tile([NC_ * 512, 512], F8,
                                          addr_space="Shared",
                                          name=f"ag_out{q}"))
            # chunk 3: pair A (heads 0,1) wide; pair B (heads 2,3) in halves
            ag_a3_in = dram.tile([128, 2, 512], F8, name="ag_a3_in")
            ag_a3_out = dram.tile([NC_ * 256, 512], F8, addr_space="Shared",
                                  name="ag_a3_out")
            ag_b3_in = dram.tile([128, 2, 512], F8, name="ag_b3_in")
            ag_b3_out = dram.tile([NC_ * 256, 512], F8, addr_space="Shared",
                                  name="ag_b3_out")
            ag2_in_c, ag2_out_c = [], []
            for icc in range(IC):
                ag2_in_c.append(dram.tile([GS, 512], F8, name=f"ag2_in{icc}"))
                ag2_out_c.append(dram.tile([GH, 512], F8, addr_space="Shared",
                                           name=f"ag2_out{icc}"))

            # one tiny warmup to kick off ncfw init (~100us) early; per-shape
            # warmups measurably do nothing for later collectives' speed
            warm_i = dram.tile([1, 512], F8, name="warm_i")
            warm_o = dram.tile([NC_, 512], F8, addr_space="Shared",
                               name="warm_o")
            nc.gpsimd.collective_compute(
                "AllGather", mybir.AluOpType.bypass, replica_groups=groups,
                ins=[warm_i.opt()], outs=[warm_o.opt()],
            )

            with tc.tile_pool(name="main", bufs=1) as mp:
                def load_xt(q):
                    xt_q = mp.tile([128, KT_TILES, 512], F8, tag="xtb",
                                   bufs=2, name="xt_q")
                    nc.sync.dma_start(out=xt_q, in_=xt_d[:, q])
                    return xt_q

                def proj_dr(wsb, msl, xt_q, ps):
                    for k in range(KP):
                        nc.tensor.matmul(
                            ps,
                            lhsT=wsb[:, 2 * k:2 * k + 2, msl],
                            rhs=xt_q[:, 2 * k:2 * k + 2, :],
                            start=(k == 0),
                            stop=(k == KP - 1),
                            perf_mode=DR,
                        )

                # ======== K/V/g1X pass (quarters 1,2,3,0) + Q(0) ========
                with tc.tile_pool(name="pkv", bufs=1) as pkv:
                    wk_sb = pkv.tile([128, KT_TILES, HS], F8, tag="wk", bufs=1)
                    wv_sb = pkv.tile([128, KT_TILES, HS], F8, tag="wv", bufs=1)

                    # first quarter (q=1) X and Wk in 4-k-tile chunks so the
                    # k-outer loop starts as soon as the first 256KB lands
                    xt_first = mp.tile([128, KT_TILES, 512], F8, tag="xtb",
                                       bufs=2, name="xt_q")
                    NCH = 8
                    for ch in range(NCH):
                        ksl = slice(ch * 4, (ch + 1) * 4)
                        nc.sync.dma_start(out=xt_first[:, ksl, :],
                                          in_=xt_d[:, 1, ksl, :])
                        nc.sync.dma_start(out=wk_sb[:, ksl, :],
                                          in_=wk_d[:, ksl, :])
                    xt_next = load_xt(2)

                    # weights on the scalar DMA queue
                    nc.scalar.dma_start(out=wv_sb, in_=wv_d)
                    nc.scalar.dma_start(out=wq_sb, in_=wq_d)
                    nc.scalar.dma_start(out=gw1x_sb, in_=gw1x_d)
                    nc.scalar.dma_start(out=wo_sb, in_=wo_d)
                    nc.scalar.dma_start(out=gwf_sb, in_=gwf_d)
                    nc.scalar.dma_start(out=gw2_sb, in_=gw2_d)
                    if not allones:
                        nc.scalar.dma_start(out=mask01_sb, in_=mask01_d)
                    nc.scalar.dma_start(out=diagm_sb, in_=diagm_d)
                    nc.scalar.dma_start(out=gb1_sb, in_=gb1_d)
                    nc.scalar.dma_start(out=gb2_sb, in_=gb2_d)

                    def v_pass(q, xt_q):
                        for it in range(4):
                            ps = psum.tile([128, 512], F32, tag="mm", bufs=2,
                                           name="ps_v")
                            for k in range(KP):
                                nc.tensor.matmul(
                                    ps,
                                    lhsT=xt_q[:, 2 * k:2 * k + 2,
                                              it * 128:(it + 1) * 128],
                                    rhs=wv_sb[:, 2 * k:2 * k + 2, :],
                                    start=(k == 0),
                                    stop=(k == KP - 1),
                                    perf_mode=DR,
                                )
                            nc.vector.tensor_scalar_mul(
                                v_sb[:, q * 4 + it, :], ps, 1.0 / W_SCALE
                            )

                    def g1x_pass(q, xt_q):
                        isl = slice(q * 512, (q + 1) * 512)
                        ps = psum.tile([128, 512], F32, tag="mm", bufs=2,
                                       name="ps_g1x")
                        proj_dr(gw1x_sb, slice(0, GS), xt_q, ps)
                        nc.vector.tensor_scalar_mul(
                            g1x_sb[:, isl], ps, 1.0 / W_SCALE
                        )

                    # quarter 1: k-outer K pass over 4 live PSUM banks
                    # (borrowing the scores' "st" tag banks)
                    kpsA = psum.tile([128, 2, 512], F32, tag="st", bufs=2,
                                     name="kpsA")
                    kpsB = psum.tile([128, 2, 512], F32, tag="st", bufs=2,
                                     name="kpsB")
                    kps = [kpsA[:, 0, :], kpsA[:, 1, :],
                           kpsB[:, 0, :], kpsB[:, 1, :]]
                    for ch in range(NCH):
                        for m in range(4):
                            for u in range(2):
                                nc.tensor.matmul(
                                    kps[m],
                                    lhsT=wk_sb[:, 4 * ch + 2 * u:
                                               4 * ch + 2 * u + 2,
                                               m * 128:(m + 1) * 128],
                                    rhs=xt_first[:, 4 * ch + 2 * u:
                                                 4 * ch + 2 * u + 2, :],
                                    start=(ch == 0 and u == 0),
                                    stop=(ch == NCH - 1 and u == 1),
                                    perf_mode=DR,
                                )
                    for m in range(4):
                        nc.vector.tensor_scalar_mul(
                            kt_sb[:, m, 512:1024], kps[m], 1.0 / W_SCALE
                        )
                    v_pass(1, xt_first)
                    g1x_pass(1, xt_first)

                    xt_last = None
                    for q in (2, 3, 0):
                        xt_q = xt_next
                        if q == 2:
                            xt_next = load_xt(3)
                        elif q == 3:
                            xt_next = load_xt(0)
                        isl = slice(q * 512, (q + 1) * 512)
                        for m in range(4):
                            ps = psum.tile([128, 512], F32, tag="mm", bufs=2,
                                           name="ps_k")
                            proj_dr(wk_sb, slice(m * 128, (m + 1) * 128),
                                    xt_q, ps)
                            nc.vector.tensor_scalar_mul(
                                kt_sb[:, m, isl], ps, 1.0 / W_SCALE
                            )
                        v_pass(q, xt_q)
                        g1x_pass(q, xt_q)
                        xt_last = xt_q

                    # stream next-quarter X for the in-loop Q projections
                    xt_blk = {1: load_xt(1)}
                    # Q(0) from the still-resident quarter-0 tile
                    qt_cur = mp.tile([128, HPC, 512], BF16, tag="qt", bufs=2,
                                     name="qt")
                    for m in range(4):
                        ps = psum.tile([128, 512], F32, tag="mm", bufs=2,
                                       name="ps_q")
                        proj_dr(wq_sb, slice(m * 128, (m + 1) * 128),
                                xt_last, ps)
                        nc.vector.tensor_scalar_mul(
                            qt_cur[:, m, :], ps, 1.0 / W_SCALE
                        )

                # ======== interleaved block phase ========

                def attention_block(h, q, qt):
                    pt = mp.tile([128, JT, 512], F8, tag="pt", bufs=2,
                                 name="pt")
                    for jp in range(JP):
                        st = psum.tile([128, 2, 512], F32, tag="st",
                                       bufs=2, name="st")
                        for uu in range(2):
                            j = 2 * jp + uu
                            nc.tensor.matmul(
                                st[:, uu, :],
                                lhsT=kt_sb[:, h, j * 128:(j + 1) * 128],
                                rhs=qt[:, h, :],
                                start=True,
                                stop=True,
                            )
                        nc.scalar.activation(
                            pt[:, 2 * jp:2 * jp + 2, :],
                            st,
                            mybir.ActivationFunctionType.Exp,
                            bias=ebias_sb,
                            scale=SCALE,
                        )
                        for uu in range(2):
                            j = 2 * jp + uu
                            if not allones:
                                nc.vector.tensor_scalar_mul(
                                    pt[:, j, :], pt[:, j, :],
                                    mask01_sb[:, j:j + 1],
                                )
                            if j // 4 == q:
                                c0 = (j % 4) * 128
                                nc.vector.tensor_mul(
                                    pt[:, j, c0:c0 + 128],
                                    pt[:, j, c0:c0 + 128],
                                    diagm_sb,
                                )
                    den_ps = psum.tile([128, 512], F32, tag="acc", bufs=2,
                                       name="den_ps")
                    for jp in range(JP):
                        nc.tensor.matmul(
                            den_ps,
                            lhsT=ones_sb,
                            rhs=pt[:, 2 * jp:2 * jp + 2, :],
                            start=(jp == 0),
                            stop=(jp == JP - 1),
                            perf_mode=DR,
                        )
                    ot_ps = psum.tile([128, 512], F32, tag="acc", bufs=2,
                                      name="ot_ps")
                    for jp in range(JP):
                        nc.tensor.matmul(
                            ot_ps,
                            lhsT=v_sb[:, 2 * jp:2 * jp + 2,
                                      h * 128:(h + 1) * 128],
                            rhs=pt[:, 2 * jp:2 * jp + 2, :],
                            start=(jp == 0),
                            stop=(jp == JP - 1),
                            perf_mode=DR,
                        )
                    rec = mp.tile([128, 512], F32, tag="rec", bufs=1)
                    nc.vector.reciprocal_approx_fast(out=rec, in_=den_ps)
                    otc = mp.tile([128, 512], F8, tag="otc", bufs=1)
                    nc.vector.tensor_mul(otc, ot_ps, rec)
                    if q < IC - 1:
                        nc.sync.dma_start(out=ag_in_c[q][:, h, :], in_=otc)
                        if h == 3:
                            nc.gpsimd.collective_compute(
                                "AllGather", mybir.AluOpType.bypass,
                                replica_groups=groups,
                                ins=[ag_in_c[q].opt()],
                                outs=[ag_out_c[q].opt()],
                            )
                    elif h < 2:
                        nc.sync.dma_start(out=ag_a3_in[:, h, :], in_=otc)
                        if h == 1:
                            nc.gpsimd.collective_compute(
                                "AllGather", mybir.AluOpType.bypass,
                                replica_groups=groups,
                                ins=[ag_a3_in.opt()],
                                outs=[ag_a3_out.opt()],
                            )
                    else:
                        nc.sync.dma_start(out=ag_b3_in[:, h - 2, :], in_=otc)
                        if h == 3:
                            nc.gpsimd.collective_compute(
                                "AllGather", mybir.AluOpType.bypass,
                                replica_groups=groups,
                                ins=[ag_b3_in.opt()],
                                outs=[ag_b3_out.opt()],
                            )

                def otg_load(ic):
                    otg = mp.tile([128, NC_, HPC, 512], F8, tag="otg", bufs=1,
                                  name="otg")
                    nc.sync.dma_start(
                        out=otg,
                        in_=ag_out_c[ic].rearrange("(r j h) i -> j r h i",
                                                   r=NC_, j=128, h=HPC),
                    )
                    return otg

                def otg_pair(otg, t):
                    # head-tile pair (2t, 2t+1) of a 4-D gathered buffer
                    return otg[:, t // 2, 2 * (t % 2):2 * (t % 2) + 2, :]

                def outproj_chunk(ic, otg):
                    csl = slice(ic * 512, (ic + 1) * 512)
                    for m in range(4):
                        ps = psum.tile([128, 512], F32, tag="mm", bufs=2,
                                       name="ps_wo")
                        for r in range(KP):
                            nc.tensor.matmul(
                                ps,
                                lhsT=wo_sb[:, 2 * r:2 * r + 2,
                                           m * 128:(m + 1) * 128],
                                rhs=otg_pair(otg, r),
                                start=(r == 0),
                                stop=(r == KP - 1),
                                perf_mode=DR,
                            )
                        nc.vector.tensor_scalar_mul(
                            cacc[:, m, csl], ps, 1.0 / (W_SCALE * O_SCALE)
                        )

                def g1_finish(pre, w, ag2i, ag2o):
                    gt_ch = mp.tile([128, w], F8, tag="gt", bufs=1)
                    nc.scalar.activation(gt_ch, pre, GELU_FUNC,
                                         bias=gb1_sb, scale=1.0)
                    nc.sync.dma_start(out=ag2i, in_=gt_ch)
                    nc.gpsimd.collective_compute(
                        "AllGather", mybir.AluOpType.bypass,
                        replica_groups=groups,
                        ins=[ag2i.opt()], outs=[ag2o.opt()],
                    )

                def g1_chunk(ic, otg):
                    csl = slice(ic * 512, (ic + 1) * 512)
                    ps = psum.tile([128, 512], F32, tag="mm", bufs=2,
                                   name="ps_g1")
                    for r in range(KP):
                        nc.tensor.matmul(
                            ps,
                            lhsT=gwf_sb[:, 2 * r:2 * r + 2, :],
                            rhs=otg_pair(otg, r),
                            start=(r == 0),
                            stop=(r == KP - 1),
                            perf_mode=DR,
                        )
                    g1pre = mp.tile([128, 512], BF16, tag="g1pre", bufs=1)
                    nc.vector.scalar_tensor_tensor(
                        g1pre, ps, 1.0 / (W_SCALE * O_SCALE),
                        g1x_sb[:, csl],
                        op0=mybir.AluOpType.mult,
                        op1=mybir.AluOpType.add,
                    )
                    g1_finish(g1pre, 512, ag2_in_c[ic], ag2_out_c[ic])

                def gtf_load(ic):
                    gtf = mp.tile([128, NC_, 512], F8, tag="gtf",
                                  bufs=1, name="gtf")
                    nc.sync.dma_start(
                        out=gtf,
                        in_=ag2_out_c[ic].rearrange("(r p) i -> p r i", p=128),
                    )
                    return gtf

                def gate_chain(c0, w, gtf):
                    csl = slice(c0, c0 + w)
                    for m in range(4):
                        ps = psum.tile([128, w], F32, tag="mm", bufs=2,
                                       name="ps_gw2")
                        for r in range(NC_ // 2):
                            nc.tensor.matmul(
                                ps,
                                lhsT=gw2_sb[:, 2 * r:2 * r + 2,
                                            m * 128:(m + 1) * 128],
                                rhs=gtf[:, 2 * r:2 * r + 2, :],
                                start=(r == 0),
                                stop=(r == NC_ // 2 - 1),
                                perf_mode=DR,
                            )
                        gate_ch = mp.tile([128, w], BF16, tag="gate",
                                          bufs=2)
                        nc.scalar.activation(
                            gate_ch, ps,
                            mybir.ActivationFunctionType.Sigmoid,
                            bias=gb2_sb[:, m:m + 1], scale=1.0 / W_SCALE,
                        )
                        outt = mp.tile([128, w], BF16, tag="outt", bufs=1)
                        nc.vector.tensor_mul(outt, gate_ch, cacc[:, m, csl])
                        nc.sync.dma_start(
                            out=out_d[m * 128:(m + 1) * 128, csl], in_=outt
                        )

                def qproj_blk(m, xtb, qt_next):
                    ps = psum.tile([128, 512], F32, tag="mm", bufs=2,
                                   name="ps_q")
                    proj_dr(wq_sb, slice(m * 128, (m + 1) * 128), xtb, ps)
                    nc.vector.tensor_scalar_mul(
                        qt_next[:, m, :], ps, 1.0 / W_SCALE
                    )

                # schedule (chunk-AG ~29us ~ 1.6 blocks):
                #   AG(ic) trigger @ 4ic+3 (in attention)
                #   otg(ic) @ 4ic+5   g1(ic) @ 4ic+6-pre  outproj(ic) @ 4ic+6
                #   gtf(ic) @ 4ic+8   gate(ic) @ 4ic+9
                otg_pend = {}
                gtf_pend = {}
                qt_next = None
                otg_a3 = None
                for s in range(16):
                    h, q = s % 4, s // 4
                    r4 = s % 4
                    attention_block(h, q, qt_cur)
                    if r4 == 0 and q + 1 < IC:
                        qt_next = mp.tile([128, HPC, 512], BF16, tag="qt",
                                          bufs=2, name="qt")
                        if q + 2 < IC:
                            xt_blk[q + 2] = load_xt(q + 2)
                    # chunk 0's consumers lag 2 extra blocks: the first AG
                    # absorbs the cross-core skew accumulated over the
                    # projection pass (~25us)
                    if s in (8, 9, 13):
                        ic = 0 if s == 8 else (s - 5) // 4
                        otg_pend[ic] = otg_load(ic)
                    if s in (9, 10, 14):
                        ic = 0 if s == 9 else (s - 6) // 4
                        g1_chunk(ic, otg_pend[ic])
                        outproj_chunk(ic, otg_pend.pop(ic))
                    if s in (11, 12):
                        ic = 0 if s == 11 else (s - 8) // 4
                        gtf_pend[ic] = gtf_load(ic)
                    if s in (12, 13):
                        ic = 0 if s == 12 else (s - 9) // 4
                        gate_chain(ic * 512, 512, gtf_pend.pop(ic))
                    if s == 14:
                        # pair-A gather of chunk 3 (triggered end of s13)
                        otg_a3 = mp.tile([128, NC_, 2, 512], F8, tag="otga3",
                                         bufs=1, name="otga3")
                        nc.sync.dma_start(
                            out=otg_a3,
                            in_=ag_a3_out.rearrange("(r j u) i -> j r u i",
                                                    r=NC_, j=128, u=2),
                        )
                    if q + 1 < IC:
                        qproj_blk(h, xt_blk[q + 1], qt_next)
                    if r4 == 3 and q + 1 < IC:
                        qt_cur = qt_next

                # ---- tail: chunk 3 (pair A wide, pair B split in halves) ----
                # pair-A out_proj and g1 A-parts run under the pair-B AG wire
                for m in range(4):
                    ps = psum.tile([128, 512], F32, tag="mm", bufs=2,
                                   name="ps_wo")
                    for r in range(NC_):
                        nc.tensor.matmul(
                            ps,
                            lhsT=wo_sb[:, 4 * r:4 * r + 2,
                                       m * 128:(m + 1) * 128],
                            rhs=otg_a3[:, r, :, :],
                            start=(r == 0),
                            stop=(r == NC_ - 1),
                            perf_mode=DR,
                        )
                    nc.vector.tensor_scalar_mul(
                        cacc[:, m, 1536:2048], ps, 1.0 / (W_SCALE * O_SCALE)
                    )
                psA = psum.tile([128, 512], F32, tag="mm", bufs=2,
                                name="ps_g1a")
                for r in range(NC_):
                    nc.tensor.matmul(
                        psA,
                        lhsT=gwf_sb[:, 4 * r:4 * r + 2, :],
                        rhs=otg_a3[:, r, :, :],
                        start=(r == 0),
                        stop=(r == NC_ - 1),
                        perf_mode=DR,
                    )
                tmpA = mp.tile([128, 512], BF16, tag="g1tmp", bufs=1)
                nc.vector.scalar_tensor_tensor(
                    tmpA, psA, 1.0 / (W_SCALE * O_SCALE),
                    g1x_sb[:, 1536:2048],
                    op0=mybir.AluOpType.mult,
                    op1=mybir.AluOpType.add,
                )
                gtf_pend[2] = gtf_load(2)
                gate_chain(1024, 512, gtf_pend.pop(2))
                otg_b = mp.tile([128, NC_, 2, 512], F8, tag="otga3",
                                bufs=1, name="otgb3")
                nc.sync.dma_start(
                    out=otg_b,
                    in_=ag_b3_out.rearrange("(r j u) i -> j r u i",
                                            r=NC_, j=128, u=2),
                )
                # g1 B-part first: it gates the AG2(3) trigger
                psb = psum.tile([128, 512], F32, tag="mm", bufs=2,
                                name="ps_g1b")
                for r in range(NC_):
                    nc.tensor.matmul(
                        psb,
                        lhsT=gwf_sb[:, 4 * r + 2:4 * r + 4, :],
                        rhs=otg_b[:, r, :, :],
                        start=(r == 0),
                        stop=(r == NC_ - 1),
                        perf_mode=DR,
                    )
                g1pre = mp.tile([128, 512], BF16, tag="g1pre", bufs=1)
                nc.vector.scalar_tensor_tensor(
                    g1pre, psb, 1.0 / (W_SCALE * O_SCALE), tmpA,
                    op0=mybir.AluOpType.mult,
                    op1=mybir.AluOpType.add,
                )
                g1_finish(g1pre, 512, ag2_in_c[3], ag2_out_c[3])
                for m in range(4):
                    ps = psum.tile([128, 512], F32, tag="mm", bufs=2,
                                   name="ps_wob")
                    for r in range(NC_):
                        nc.tensor.matmul(
                            ps,
                            lhsT=wo_sb[:, 4 * r + 2:4 * r + 4,
                                       m * 128:(m + 1) * 128],
                            rhs=otg_b[:, r, :, :],
                            start=(r == 0),
                            stop=(r == NC_ - 1),
                            perf_mode=DR,
                        )
                    nc.vector.scalar_tensor_tensor(
                        cacc[:, m, 1536:2048], ps,
                        1.0 / (W_SCALE * O_SCALE),
                        cacc[:, m, 1536:2048],
                        op0=mybir.AluOpType.mult,
                        op1=mybir.AluOpType.add,
                    )
                gtf3 = gtf_load(3)
                gate_chain(1536, 512, gtf3)

    nc.compile()
    return nc


def _q8(x, scale=1.0):
    f8 = ml_dtypes.float8_e4m3
    return np.ascontiguousarray(
        np.clip(np.asarray(x, dtype=np.float32) * scale, -240.0, 240.0)
    ).astype(f8)


def _make_in_maps(inputs):
    f32 = np.float32
    X = np.asarray(inputs["hidden_states"], dtype=f32)
    mask = np.asarray(inputs["attention_mask"])
    Wq = np.asarray(inputs["Wq"], dtype=f32)
    Wk = np.asarray(inputs["Wk"], dtype=f32)
    Wv = np.asarray(inputs["Wv"], dtype=f32)
    Wo = np.asarray(inputs["Wo"], dtype=f32)
    gW1 = np.asarray(inputs["gW1"], dtype=f32)
    gb1 = np.asarray(inputs["gb1"], dtype=f32)
    gW2 = np.asarray(inputs["gW2"], dtype=f32)
    gb2 = np.asarray(inputs["gb2"], dtype=f32)

    XT8 = _q8(X.T)                                       # [4096, 2048]
    XTT = np.ascontiguousarray(
        XT8.reshape(KT_TILES, 128, IC, 512).transpose(1, 2, 0, 3))

    def _tile_w(w8):  # [K, M] -> [128, K/128, M]
        kt = w8.shape[0] // 128
        return np.ascontiguousarray(
            w8.reshape(kt, 128, w8.shape[1]).transpose(1, 0, 2))

    mask01_t = np.ascontiguousarray(
        mask.astype(f32).reshape(JT, 128).T).astype(ml_dtypes.bfloat16)
    diagm = _q8(1.0 - np.eye(128, dtype=f32))

    # fused Wo @ gW1c: attention-output features (natural head order) -> gh
    Wf = Wo @ gW1[HID:]                                  # [4096, 1024]
    gW1x = gW1[:HID]                                     # [4096, 1024]

    in_maps = []
    for c in range(NC_):
        hsl = slice(c * HS, (c + 1) * HS)
        gsl = slice(c * GS, (c + 1) * GS)
        in_maps.append({
            "xt": XTT,
            "wq": _tile_w(_q8(Wq[:, hsl], W_SCALE)),
            "wk": _tile_w(_q8(Wk[:, hsl], W_SCALE)),
            "wv": _tile_w(_q8(Wv[:, hsl], W_SCALE)),
            "wo": _tile_w(_q8(Wo[:, hsl], W_SCALE)),
            "gw1x": _tile_w(_q8(gW1x[:, gsl], W_SCALE)),
            "gwf": _tile_w(_q8(Wf[:, gsl], W_SCALE)),
            "gw2": _tile_w(_q8(gW2[:, hsl], W_SCALE)),
            "gb1": np.ascontiguousarray(gb1[gsl].reshape(GS, 1)),
            "gb2": np.ascontiguousarray(gb2[hsl].reshape(4, 128).T),
            "mask01": mask01_t,
            "diagm": diagm,
        })
    return in_maps


_NC_CACHE = {}


def _run(inputs, trace=False):
    allones = bool(np.asarray(inputs["attention_mask"]).all())
    nc = _NC_CACHE.get(allones)
    if nc is None:
        nc = _build_program(allones)
        _NC_CACHE[allones] = nc
    in_maps = _make_in_maps(inputs)
    res = bass_utils.run_bass_kernel_spmd(
        nc, in_maps, core_ids=list(range(NC_)), trace=trace
    )
    shards = [np.asarray(res.results[c]["out"], dtype=np.float32)
              for c in range(NC_)]
    gated = np.concatenate(shards, axis=0).T  # gate * cross, [2048, 4096]
    out = np.asarray(inputs["hidden_states"], dtype=np.float32) + gated
    return np.ascontiguousarray(out), res


def kernel(**inputs) -> np.ndarray:
    out, _ = _run(inputs, trace=False)
    return out
